# revision 20
# baseline (speedup 1.0000x reference)
"""Trainium2 Bass kernel for CartesianLoss (v6, compact fold + merged arms).

Loss = mean_n min_perm mean_i ||polar2cart(target_i) - polar2cart(pred_perm(i))||_2

Pure data parallelism over the batch (N=131072) across 8 cores; each core
handles 16384 samples as (128 partitions, 128 samples). Host packs inputs
chunk-major, source-major, fp16, so every device op is contiguous fp16
(DVE 2x packed mode) with no on-device transposes.

v6 vs v3:
- the triangle fold writes COMPACT pair tiles F[f, k, s] (k = 10 unordered
  pairs) instead of folding in place, so the combine is ONE tensor_tensor
  (compact F01 + rank-reversed G3) and arm reads use compact strides.
- Ar/Aq arm ops with consecutive compact-F23 ranges merge pairwise via an
  arm-dim stride (12 -> 10 arm ops).
- split ang/dst input DMAs: trig starts one transfer earlier.

HW-validated dead ends (defaults keep them off): GpSimd offload of any
elementwise op costs ~+7us real (SBUF-port contention with DVE; the cost
model doesn't see it). tensor_tensor_reduce tail fusion wedges the device
(CoreSim-correct, NEFF execution fails). The bias-via-DMA preamble trick
and multi-engine DMA issue both lose in the timeline model.

Assignment (min over 120 perms) uses meet-in-the-middle over targets
{0,1} | {2,3} | {4}: F01/F23 pair mins via dense 5x5 outer-sum + compact
triangle fold, g3 triples via 3 arms, combine with reversed-rank access
(complement of the k-th pair is the (9-k)-th sorted triple).
"""

import contextlib

import numpy as np

import concourse.bass as bass
import concourse.bass_isa as bass_isa
import concourse.bacc as bacc
import concourse.tile as tile
from concourse import mybir

N = 131072
M = 5
NCORES = 8
NPC = N // NCORES          # samples per core
P = 128                    # partitions
FS = NPC // P              # samples per partition (128)
HALF_PI = 1.5707963267948966

F32 = mybir.dt.float32
F16 = mybir.dt.float16
TT = mybir.AluOpType
AFT = mybir.ActivationFunctionType

# --- tunables -------------------------------------------------------------
NCH = 2                    # front-end sample chunks (divides FS)
SQ_ENGINE = "split"        # 'act' | 'dve' | 'split' (squares of dx/dy)
D2_ENGINE = "dve"          # 'dve' | 'gp'    (d2 = dx2 + dy2)
ARMT_ENGINE = "dve"        # 'dve' | 'gp'    (arm_t adds)
G3_ENGINE = "dve"          # 'dve' | 'gp'    (3-way arm min)
DMA_SPLIT = True           # separate ang/dst DMAs (trig starts earlier)
EARLY_SQRT_LOAD = False     # dummy sqrt after trig to hoist the table load

TRACE = False
USE_TTR = False            # tensor_tensor_reduce tail fusion (breaks on HW)
BIAS_DMA = False           # half-pi trig bias via input DMA instead of
                           # memset + all-engine barrier in the preamble
DMA_SPREAD = False         # issue input DMAs from SP/ACT/DVE queues

ROWSTART = (0, 4, 7, 9)    # compact pair index: (a,b),a<b -> ROWSTART[a]+b-a-1


def _ap(t, offset_elems, dims):
    """Manual free-dim AP on tile t: dims = [[step,count],...] (elements)."""
    full = t[:]
    return bass.AP(
        tensor=full.tensor,
        offset=full.offset + offset_elems,
        ap=[full.ap[0]] + [list(d) for d in dims],
    )


def build_bass(loop_iters=None, nch=None, sq_engine=None, d2_engine=None,
               armt_engine=None, g3_engine=None, dma_split=None,
               early_sqrt_load=None):
    nch = NCH if nch is None else nch
    sq_engine = SQ_ENGINE if sq_engine is None else sq_engine
    d2_engine = D2_ENGINE if d2_engine is None else d2_engine
    armt_engine = ARMT_ENGINE if armt_engine is None else armt_engine
    g3_engine = G3_ENGINE if g3_engine is None else g3_engine
    dma_split = DMA_SPLIT if dma_split is None else dma_split
    early_sqrt_load = (EARLY_SQRT_LOAD if early_sqrt_load is None
                       else early_sqrt_load)
    CS = FS // nch
    W = FS
    assert FS % nch == 0

    nc = bacc.Bacc(
        "TRN2", target_bir_lowering=False, debug=False, num_devices=NCORES
    )
    if not BIAS_DMA:
        hpi_t = nc.alloc_sbuf_tensor("const-float32-hpi", [P, 1], F32)
        nc.gpsimd.memset(hpi_t.ap(), HALF_PI)
        nc.const_aps.aps[(F32, HALF_PI)] = hpi_t.ap()
        nc.all_engine_barrier()

    in_d = nc.dram_tensor("inp", [P, nch, 2, 2, M, CS], F16, kind="ExternalInput")
    if BIAS_DMA:
        hpi_d = nc.dram_tensor("hpi", [P, 1], F32, kind="ExternalInput")
    out_d = nc.dram_tensor("partials", [1, 1], F32, kind="ExternalOutput")

    gp = nc.gpsimd
    dve = nc.vector
    eng = {"dve": dve, "gp": gp}
    MCS = M * CS

    with tile.TileContext(nc) as tc:
        with contextlib.ExitStack() as stack:
            if loop_iters is not None:
                stack.enter_context(tc.For_i(0, loop_iters, 1))
            pool = stack.enter_context(tc.tile_pool(name="main", bufs=1))

            def tl(shape, dt, tag):
                return pool.tile(shape, dt, name="t", tag=tag)

            IN = [tl([P, 2, 2, M, CS], F16, f"in{c}") for c in range(nch)]
            ang = [t[:, 0] for t in IN]
            # TRIG[h]: h=0 cos, h=1 sin, each [2(t/p), M, CS]
            TRIG = [tl([P, 2, 2, M, CS], F16, f"trig{c}") for c in range(nch)]
            CRD = [tl([P, 2, 2, M, CS], F16, f"crd{c}") for c in range(nch)]
            DXY = [tl([P, 2, M, M, CS], F16, f"dxy{c}") for c in range(nch)]
            SQ = [tl([P, 2, M, M, CS], F16, f"sq{c}") for c in range(nch)]
            D2 = [tl([P, M * M, CS], F16, f"d2{c}") for c in range(nch)]
            D = tl([P, M * M, FS], F16, "dfull")
            GT = tl([P, 2, M, M, W], F16, "gt")   # dense 5x5 outer sums
            FC = tl([P, 2, 10, W], F16, "fc")     # compact pair mins
            # ARM[s]: s=0 At, s=1 Ar, s=2 Aq (adjacent so Ar/Aq ops merge)
            ARM = tl([P, 3, 10, W], F16, "arm")
            G3 = tl([P, 10, W], F16, "g3")
            ANS = tl([P, 10, W], F16, "ans")
            T1 = tl([P, M, W], F16, "t1")
            T2 = tl([P, 2, W], F16, "t2")
            T3 = tl([P, 1, W], F16, "t3")
            RES = tl([P, 1, W], F16, "res")
            SCR = tl([P, 1], F16, "scr")
            PART = tl([P, 1], F32, "part")
            PARTR = tl([P, 1], F32, "partr")

            # ---- DMA: the trig-gating transfers (hpi bias + chunk-0 angles)
            # go first on SP; the rest spread across idle engine DGE queues
            # so the transfers overlap instead of serializing on one queue.
            if BIAS_DMA:
                BIAS = tl([P, 1], F32, "bias")
                nc.sync.dma_start(out=BIAS[:], in_=hpi_d[:])
                cos_bias = BIAS[:]
            else:
                cos_bias = HALF_PI
            dma_eng = {0: nc.sync, 1: nc.scalar, 2: nc.sync, 3: nc.scalar}
            for c in range(nch):
                if dma_split:
                    e_a = dma_eng[2 * c] if DMA_SPREAD else nc.sync
                    e_d = dma_eng[2 * c + 1] if DMA_SPREAD else nc.sync
                    e_a.dma_start(out=IN[c][:, 0], in_=in_d[:, c, 0])
                    e_d.dma_start(out=IN[c][:, 1], in_=in_d[:, c, 1])
                else:
                    nc.sync.dma_start(out=IN[c][:], in_=in_d[:, c])

            # ---- ACT trig: all chunks first (one table set) ----
            for c in range(nch):
                nc.scalar.activation(TRIG[c][:, 0], ang[c], AFT.Sin, bias=cos_bias)
                nc.scalar.activation(TRIG[c][:, 1], ang[c], AFT.Sin)
            if early_sqrt_load:
                # 1-element Sqrt forces the sqrt-set table load now, while
                # ACT is otherwise idle waiting on DVE's dxy (input is the
                # initialized half-pi value so the result is finite)
                src = BIAS[:] if BIAS_DMA else hpi_t.ap()
                nc.scalar.activation(SCR[:], src, AFT.Sqrt)

            # ---- DVE front-end per chunk ----
            for c in range(nch):
                # CRD[h,tp,m,s] = TRIG[h,tp,m,s] * dst[tp,m,s]. Chunk 0 is
                # split per h so DVE starts right after the first ACT op
                # (cos0); later chunks' trig is long done, so one merged op.
                if c == 0:
                    for h in range(2):
                        dve.tensor_tensor(
                            _ap(CRD[c], h * 2 * MCS, [[MCS, 2], [1, MCS]]),
                            _ap(TRIG[c], h * 2 * MCS, [[MCS, 2], [1, MCS]]),
                            _ap(IN[c], 2 * MCS, [[MCS, 2], [1, MCS]]),
                            TT.mult,
                        )
                        dve.tensor_tensor(
                            DXY[c][:, h],
                            _ap(CRD[c], h * 2 * MCS, [[CS, M], [0, M], [1, CS]]),
                            _ap(CRD[c], h * 2 * MCS + MCS,
                                [[0, M], [CS, M], [1, CS]]),
                            TT.subtract,
                        )
                else:
                    dve.tensor_tensor(
                        _ap(CRD[c], 0, [[2 * MCS, 2], [MCS, 2], [1, MCS]]),
                        _ap(TRIG[c], 0, [[2 * MCS, 2], [MCS, 2], [1, MCS]]),
                        _ap(IN[c], 2 * MCS, [[0, 2], [MCS, 2], [1, MCS]]),
                        TT.mult,
                    )
                    for h in (1, 0):
                        dve.tensor_tensor(
                            DXY[c][:, h],
                            _ap(CRD[c], h * 2 * MCS, [[CS, M], [0, M], [1, CS]]),
                            _ap(CRD[c], h * 2 * MCS + MCS,
                                [[0, M], [CS, M], [1, CS]]),
                            TT.subtract,
                        )
                if sq_engine == "dve":
                    dve.tensor_tensor(SQ[c][:], DXY[c][:], DXY[c][:], TT.mult)

            # ---- squares; d2; sqrt in row groups (0-9 gates F01 pairs,
            #      10-19 gates F23 pairs, 20-24 gates arms) ----
            for c in range(nch):
                if sq_engine == "act":
                    nc.scalar.activation(SQ[c][:], DXY[c][:], AFT.Square)
                elif sq_engine == "split":
                    nc.scalar.activation(SQ[c][:, 1], DXY[c][:, 1], AFT.Square)
            for c in range(nch):
                if sq_engine == "split":
                    dve.tensor_tensor(
                        SQ[c][:, 0], DXY[c][:, 0], DXY[c][:, 0], TT.mult
                    )
                eng[d2_engine].tensor_tensor(
                    D2[c][:], SQ[c][:, 0], SQ[c][:, 1], TT.add
                )
            for r0, nrows in ((0, 10), (10, 10), (20, 5)):
                for c in range(nch):
                    nc.scalar.activation(
                        _ap(D, r0 * FS + c * CS, [[FS, nrows], [1, CS]]),
                        D2[c][:, r0:r0 + nrows], AFT.Sqrt,
                    )

            # ---- dense outer-sums: GT[f,a,b,s] = D[r0(f),a,s] + D[r1(f),b,s]
            # rows (0,1) for f=0 -> F01, rows (2,3) for f=1 -> F23. Split by
            # sample-half aligned to the sqrt chunks so each half starts as
            # soon as its chunk's rows are sqrted. ----
            HW_ = W // 2
            for f, (r0, r1) in enumerate(((0, 1), (2, 3))):
                for ho in (0, HW_):
                    dve.tensor_tensor(
                        _ap(GT, f * 25 * W + ho,
                            [[M * W, M], [W, M], [1, HW_]]),
                        _ap(D, r0 * 5 * FS + ho,
                            [[FS, M], [0, M], [1, HW_]]),
                        _ap(D, r1 * 5 * FS + ho,
                            [[0, M], [FS, M], [1, HW_]]),
                        TT.add,
                    )
            # compact dual-f triangle fold: FC[f,k] = min(G[a,b], G[b,a])
            for a in range(4):
                n = 4 - a
                dve.tensor_tensor(
                    _ap(FC, ROWSTART[a] * W, [[10 * W, 2], [W, n], [1, W]]),
                    _ap(GT, ((a * M) + a + 1) * W,
                        [[25 * W, 2], [W, n], [1, W]]),
                    _ap(GT, ((a + 1) * M + a) * W,
                        [[25 * W, 2], [M * W, n], [1, W]]),
                    TT.min,
                )

            # ---- arms: G3[T] (T-sorted 3-subsets) = min over c in T of
            # F23[T\c] + D4[c]; At computed first, consumed last, so a slow
            # engine there hides behind DVE's Ar/Aq. ----
            def f23c(idx, dims):
                return _ap(FC, (10 + idx) * W, dims)

            def d4(j, dims):
                return _ap(D, (20 + j) * FS, dims)

            e_t = eng[armt_engine]
            At_o = 0            # ARM slot offsets (elements)
            Ar_o = 10 * W
            # arm_t: At[T] = F23[{q,r}] + D4[t]
            e_t.tensor_tensor(
                _ap(ARM, At_o, [[W, 3], [1, W]]),
                f23c(0, [[0, 3], [1, W]]),
                d4(2, [[FS, 3], [1, W]]), TT.add)
            e_t.tensor_tensor(
                _ap(ARM, At_o + 3 * W, [[3 * W, 2], [W, 2], [1, W]]),
                f23c(1, [[3 * W, 2], [0, 2], [1, W]]),
                d4(3, [[0, 2], [FS, 2], [1, W]]), TT.add)
            e_t.tensor_tensor(
                _ap(ARM, At_o + 5 * W, [[3 * W, 2], [1, W]]),
                f23c(2, [[3 * W, 2], [1, W]]),
                d4(4, [[0, 2], [1, W]]), TT.add)
            e_t.tensor_tensor(
                _ap(ARM, At_o + 9 * W, [[0, 1], [1, W]]),
                f23c(7, [[0, 1], [1, W]]),
                d4(4, [[0, 1], [1, W]]), TT.add)
            # arm_r: Ar[T] = F23[{q,t}] + D4[r];  arm_q: Aq[T] = F23[{r,t}]
            # + D4[q]. Ar/Aq slot-0-2 and slot-9 ops read consecutive f23c
            # ranges (1-3 | 4-6 and 8 | 9), so each pair merges into one op
            # via an arm-dim stride.
            dve.tensor_tensor(
                _ap(ARM, Ar_o, [[10 * W, 2], [W, 3], [1, W]]),
                f23c(1, [[3 * W, 2], [W, 3], [1, W]]),
                d4(1, [[-FS, 2], [0, 3], [1, W]]), TT.add)
            dve.tensor_tensor(
                _ap(ARM, Ar_o + 3 * W, [[3 * W, 2], [W, 2], [1, W]]),
                f23c(2, [[3 * W, 2], [W, 2], [1, W]]),
                d4(2, [[0, 2], [0, 2], [1, W]]), TT.add)
            dve.tensor_tensor(
                _ap(ARM, Ar_o + 5 * W, [[3 * W, 2], [1, W]]),
                f23c(3, [[3 * W, 2], [1, W]]),
                d4(3, [[0, 2], [1, W]]), TT.add)
            dve.tensor_tensor(
                _ap(ARM, Ar_o + 9 * W, [[10 * W, 2], [1, W]]),
                f23c(8, [[W, 2], [1, W]]),
                d4(3, [[-FS, 2], [1, W]]), TT.add)
            # remaining Aq-only ops (slots 3,4,6,7 and 5,8)
            dve.tensor_tensor(
                _ap(ARM, Ar_o + 10 * W + 3 * W, [[3 * W, 2], [W, 2], [1, W]]),
                f23c(7, [[0, 2], [W, 2], [1, W]]),
                d4(0, [[FS, 2], [0, 2], [1, W]]), TT.add)
            dve.tensor_tensor(
                _ap(ARM, Ar_o + 10 * W + 5 * W, [[3 * W, 2], [1, W]]),
                f23c(9, [[0, 2], [1, W]]),
                d4(0, [[FS, 2], [1, W]]), TT.add)

            g3e = eng[g3_engine]
            g3e.tensor_tensor(G3[:], ARM[:, 1], ARM[:, 2], TT.min)
            g3e.tensor_tensor(G3[:], G3[:], ARM[:, 0], TT.min)

            # combine: ANS[k] = F01c[k] + G3[9-k] (complement reversal)
            dve.tensor_tensor(
                ANS[:], _ap(FC, 0, [[W, 10], [1, W]]),
                _ap(G3, 9 * W, [[-W, 10], [1, W]]), TT.add)
            # min tree over the 10 slots
            dve.tensor_tensor(T1[:], ANS[:, 0:5, :], ANS[:, 5:10, :], TT.min)
            dve.tensor_tensor(T2[:], T1[:, 0:2, :], T1[:, 2:4, :], TT.min)
            dve.tensor_tensor(T3[:], T2[:, 0:1, :], T2[:, 1:2, :], TT.min)
            if USE_TTR:
                # fused: RES = min(T3, T1[4]); PART = sum_s RES (one DVE op)
                dve.tensor_tensor_reduce(
                    RES[:, 0], T3[:, 0, :], T1[:, 4, :], 1.0, 0.0,
                    TT.min, TT.add, PART[:],
                )
            else:
                dve.tensor_tensor(RES[:, 0], T3[:, 0, :], T1[:, 4, :], TT.min)
                dve.tensor_reduce(
                    PART[:], _ap(RES, 0, [[1, W]]),
                    mybir.AxisListType.X, TT.add,
                )
            # collapse to one partition so the output DMA is a single-queue
            # 4-byte transfer (a [128,1] source fans out over 16 queues whose
            # completion crawl costs ~7us at the tail)
            gp.partition_all_reduce(
                PARTR[:], PART[:], 128, bass_isa.ReduceOp.add
            )
            nc.sync.dma_start(out=out_d[:], in_=PARTR[0:1, :])

    nc.compile()
    return nc


_CACHED_RUNNER = None


def _pack_inputs(ta, pa, td, pd, nch):
    """(N, M) f32 x4 -> (NCORES*P, nch, 2, 2, M, CS) f16, chunk-major."""
    CS = FS // nch
    out = np.empty((NCORES * P, nch, 2, 2, M, CS), np.float16)
    for k, (a, b) in enumerate(((ta, pa), (td, pd))):
        a5 = np.asarray(a, np.float32).reshape(NCORES * P, nch, CS, M)
        b5 = np.asarray(b, np.float32).reshape(NCORES * P, nch, CS, M)
        out[:, :, k, 0] = a5.transpose(0, 1, 3, 2)
        out[:, :, k, 1] = b5.transpose(0, 1, 3, 2)
    return out


def _make_runner():
    import jax
    from jax.sharding import Mesh, NamedSharding, PartitionSpec
    from jax.experimental.shard_map import shard_map
    from concourse.bass2jax import (
        _bass_exec_p, install_neuronx_cc_hook, partition_id_tensor,
    )

    nc = build_bass()
    install_neuronx_cc_hook()
    partition_name = nc.partition_id_tensor.name if nc.partition_id_tensor else None
    in_names, out_names, out_avals, zero_outs = [], [], [], []
    for alloc in nc.m.functions[0].allocations:
        if not isinstance(alloc, mybir.MemoryLocationSet):
            continue
        name = alloc.memorylocations[0].name
        if alloc.kind == "ExternalInput":
            if name != partition_name:
                in_names.append(name)
        elif alloc.kind == "ExternalOutput":
            shape = tuple(alloc.tensor_shape)
            dtype = mybir.dt.np(alloc.dtype)
            out_names.append(name)
            out_avals.append(jax.core.ShapedArray(shape, dtype))
            zero_outs.append(np.zeros(shape, dtype))
    n_params = len(in_names)
    all_in_names = in_names + out_names
    if partition_name is not None:
        all_in_names = all_in_names + [partition_name]

    def _body(*args):
        operands = list(args)
        if partition_name is not None:
            operands.append(partition_id_tensor())
        return tuple(_bass_exec_p.bind(
            *operands,
            out_avals=tuple(out_avals),
            in_names=tuple(all_in_names),
            out_names=tuple(out_names),
            lowering_input_output_aliases=(),
            sim_require_finite=True,
            sim_require_nnan=True,
            nc=nc,
        ))

    devices = jax.devices()[:NCORES]
    mesh = Mesh(np.asarray(devices), ("core",))
    in_specs = (PartitionSpec("core"),) * (n_params + len(out_names))
    out_specs = (PartitionSpec("core"),) * len(out_names)
    fn = jax.jit(
        shard_map(_body, mesh=mesh, in_specs=in_specs, out_specs=out_specs,
                  check_rep=False),
        keep_unused=True,
    )
    sharding = NamedSharding(mesh, PartitionSpec("core"))
    concat_zeros = [
        np.zeros((NCORES * z.shape[0], *z.shape[1:]), z.dtype) for z in zero_outs
    ]
    zeros_dev = [jax.device_put(z, sharding) for z in concat_zeros]

    def run(inputs_by_name):
        import jax as _jax
        args = [
            _jax.device_put(np.ascontiguousarray(inputs_by_name[nm]), sharding)
            for nm in in_names
        ]
        outs = fn(*args, *zeros_dev)
        return {nm: np.asarray(outs[i]) for i, nm in enumerate(out_names)}

    return run


def _input_map(ta, pa, td, pd):
    m = {"inp": _pack_inputs(ta, pa, td, pd, NCH)}
    if BIAS_DMA:
        m["hpi"] = np.tile(np.full((P, 1), HALF_PI, np.float32), (NCORES, 1))
    return m


def kernel(predictions_angle, targets_angle, predictions_distance, targets_distance):
    global _CACHED_RUNNER
    if _CACHED_RUNNER is None:
        _CACHED_RUNNER = _make_runner()
    out = _CACHED_RUNNER(_input_map(
        targets_angle, predictions_angle,
        targets_distance, predictions_distance,
    ))
    total = out["partials"].astype(np.float64).sum()
    return np.asarray(total / N / M, dtype=np.float32)


# revision 23
# speedup vs baseline: 1.0699x; 1.0699x over previous
"""Trainium2 Bass kernel for CartesianLoss (v6, compact fold + merged arms).

Loss = mean_n min_perm mean_i ||polar2cart(target_i) - polar2cart(pred_perm(i))||_2

Pure data parallelism over the batch (N=131072) across 8 cores; each core
handles 16384 samples as (128 partitions, 128 samples). Host packs inputs
chunk-major, source-major, fp16, so every device op is contiguous fp16
(DVE 2x packed mode) with no on-device transposes.

v6 vs v3:
- the triangle fold writes COMPACT pair tiles F[f, k, s] (k = 10 unordered
  pairs) instead of folding in place, so the combine is ONE tensor_tensor
  (compact F01 + rank-reversed G3) and arm reads use compact strides.
- Ar/Aq arm ops with consecutive compact-F23 ranges merge pairwise via an
  arm-dim stride (12 -> 10 arm ops).
- split ang/dst input DMAs: trig starts one transfer earlier.

HW-validated dead ends (defaults keep them off): GpSimd offload of any
elementwise op costs ~+7us real (SBUF-port contention with DVE; the cost
model doesn't see it). tensor_tensor_reduce tail fusion wedges the device
(CoreSim-correct, NEFF execution fails). The bias-via-DMA preamble trick
and multi-engine DMA issue both lose in the timeline model.

Assignment (min over 120 perms) uses meet-in-the-middle over targets
{0,1} | {2,3} | {4}: F01/F23 pair mins via dense 5x5 outer-sum + compact
triangle fold, g3 triples via 3 arms, combine with reversed-rank access
(complement of the k-th pair is the (9-k)-th sorted triple).
"""

import contextlib

import numpy as np

import concourse.bass as bass
import concourse.bass_isa as bass_isa
import concourse.bacc as bacc
import concourse.tile as tile
from concourse import mybir

N = 131072
M = 5
NCORES = 8
NPC = N // NCORES          # samples per core
P = 128                    # partitions
FS = NPC // P              # samples per partition (128)
HALF_PI = 1.5707963267948966

F32 = mybir.dt.float32
F16 = mybir.dt.float16
TT = mybir.AluOpType
AFT = mybir.ActivationFunctionType

# --- tunables -------------------------------------------------------------
NCH = 2                    # front-end sample chunks (divides FS)
SQ_ENGINE = "split"        # 'act' | 'dve' | 'split' (squares of dx/dy)
D2_ENGINE = "dve"          # 'dve' | 'gp'    (d2 = dx2 + dy2)
ARMT_ENGINE = "dve"        # 'dve' | 'gp'    (arm_t adds)
G3_ENGINE = "dve"          # 'dve' | 'gp'    (3-way arm min)
DMA_SPLIT = True           # separate ang/dst DMAs (trig starts earlier)
EARLY_SQRT_LOAD = False     # dummy sqrt after trig to hoist the table load

TRACE = False
USE_TTR = False            # tensor_tensor_reduce tail fusion (breaks on HW)
BIAS_DMA = False           # half-pi trig bias via input DMA instead of
                           # memset + all-engine barrier in the preamble
DMA_SPREAD = False         # issue input DMAs from SP/ACT/DVE queues

ROWSTART = (0, 4, 7, 9)    # compact pair index: (a,b),a<b -> ROWSTART[a]+b-a-1


def _ap(t, offset_elems, dims):
    """Manual free-dim AP on tile t: dims = [[step,count],...] (elements)."""
    full = t[:]
    return bass.AP(
        tensor=full.tensor,
        offset=full.offset + offset_elems,
        ap=[full.ap[0]] + [list(d) for d in dims],
    )


def build_bass(loop_iters=None, nch=None, sq_engine=None, d2_engine=None,
               armt_engine=None, g3_engine=None, dma_split=None,
               early_sqrt_load=None, loop_staggered=False, bufs=1):
    nch = NCH if nch is None else nch
    sq_engine = SQ_ENGINE if sq_engine is None else sq_engine
    d2_engine = D2_ENGINE if d2_engine is None else d2_engine
    armt_engine = ARMT_ENGINE if armt_engine is None else armt_engine
    g3_engine = G3_ENGINE if g3_engine is None else g3_engine
    dma_split = DMA_SPLIT if dma_split is None else dma_split
    early_sqrt_load = (EARLY_SQRT_LOAD if early_sqrt_load is None
                       else early_sqrt_load)
    CS = FS // nch
    W = FS
    assert FS % nch == 0

    nc = bacc.Bacc(
        "TRN2", target_bir_lowering=False, debug=False, num_devices=NCORES
    )
    if not BIAS_DMA:
        hpi_t = nc.alloc_sbuf_tensor("const-float32-hpi", [P, 1], F32)
        nc.gpsimd.memset(hpi_t.ap(), HALF_PI)
        nc.const_aps.aps[(F32, HALF_PI)] = hpi_t.ap()
        nc.all_engine_barrier()

    in_d = nc.dram_tensor("inp", [P, nch, 2, 2, M, CS], F16, kind="ExternalInput")
    if BIAS_DMA:
        hpi_d = nc.dram_tensor("hpi", [P, 1], F32, kind="ExternalInput")
    out_d = nc.dram_tensor("partials", [1, 1], F32, kind="ExternalOutput")

    gp = nc.gpsimd
    dve = nc.vector
    eng = {"dve": dve, "gp": gp}
    MCS = M * CS

    with tile.TileContext(nc) as tc:
        with contextlib.ExitStack() as stack:
            if loop_iters is not None:
                stack.enter_context(
                    tc.For_i(0, loop_iters, 1, staggered_reset=loop_staggered)
                )
            pool = stack.enter_context(tc.tile_pool(name="main", bufs=bufs))

            def tl(shape, dt, tag):
                return pool.tile(shape, dt, name="t", tag=tag)

            IN = [tl([P, 2, 2, M, CS], F16, f"in{c}") for c in range(nch)]
            ang = [t[:, 0] for t in IN]
            # TRIG[h]: h=0 cos, h=1 sin, each [2(t/p), M, CS]
            TRIG = [tl([P, 2, 2, M, CS], F16, f"trig{c}") for c in range(nch)]
            CRD = [tl([P, 2, 2, M, CS], F16, f"crd{c}") for c in range(nch)]
            DXY = [tl([P, 2, M, M, CS], F16, f"dxy{c}") for c in range(nch)]
            SQ = [tl([P, 2, M, M, CS], F16, f"sq{c}") for c in range(nch)]
            D2 = [tl([P, M * M, CS], F16, f"d2{c}") for c in range(nch)]
            D = tl([P, M * M, FS], F16, "dfull")
            GT = tl([P, 2, M, M, W], F16, "gt")   # dense 5x5 outer sums
            FC = tl([P, 2, 10, W], F16, "fc")     # compact pair mins
            # ARM[s]: s=0 At, s=1 Ar, s=2 Aq (adjacent so Ar/Aq ops merge)
            ARM = tl([P, 3, 10, W], F16, "arm")
            G3 = tl([P, 10, W], F16, "g3")
            ANS = tl([P, 10, W], F16, "ans")
            T1 = tl([P, M, W], F16, "t1")
            T2 = tl([P, 2, W], F16, "t2")
            T3 = tl([P, 1, W], F16, "t3")
            RES = tl([P, 1, W], F16, "res")
            SCR = tl([P, 1], F16, "scr")
            PART = tl([P, 1], F32, "part")
            PARTR = tl([P, 1], F32, "partr")

            # ---- DMA: the trig-gating transfers (hpi bias + chunk-0 angles)
            # go first on SP; the rest spread across idle engine DGE queues
            # so the transfers overlap instead of serializing on one queue.
            if BIAS_DMA:
                BIAS = tl([P, 1], F32, "bias")
                nc.sync.dma_start(out=BIAS[:], in_=hpi_d[:])
                cos_bias = BIAS[:]
            else:
                cos_bias = HALF_PI
            dma_eng = {0: nc.sync, 1: nc.scalar, 2: nc.sync, 3: nc.scalar}
            for c in range(nch):
                if dma_split:
                    e_a = dma_eng[2 * c] if DMA_SPREAD else nc.sync
                    e_d = dma_eng[2 * c + 1] if DMA_SPREAD else nc.sync
                    e_a.dma_start(out=IN[c][:, 0], in_=in_d[:, c, 0])
                    e_d.dma_start(out=IN[c][:, 1], in_=in_d[:, c, 1])
                else:
                    nc.sync.dma_start(out=IN[c][:], in_=in_d[:, c])

            # ---- ACT trig: all chunks first (one table set) ----
            for c in range(nch):
                nc.scalar.activation(TRIG[c][:, 0], ang[c], AFT.Sin, bias=cos_bias)
                nc.scalar.activation(TRIG[c][:, 1], ang[c], AFT.Sin)
            if early_sqrt_load:
                # 1-element Sqrt forces the sqrt-set table load now, while
                # ACT is otherwise idle waiting on DVE's dxy (input is the
                # initialized half-pi value so the result is finite)
                src = BIAS[:] if BIAS_DMA else hpi_t.ap()
                nc.scalar.activation(SCR[:], src, AFT.Sqrt)

            # ---- DVE front-end per chunk ----
            for c in range(nch):
                # CRD[h,tp,m,s] = TRIG[h,tp,m,s] * dst[tp,m,s]. Chunk 0 is
                # split per h so DVE starts right after the first ACT op
                # (cos0); later chunks' trig is long done, so one merged op.
                if c == 0:
                    for h in range(2):
                        dve.tensor_tensor(
                            _ap(CRD[c], h * 2 * MCS, [[MCS, 2], [1, MCS]]),
                            _ap(TRIG[c], h * 2 * MCS, [[MCS, 2], [1, MCS]]),
                            _ap(IN[c], 2 * MCS, [[MCS, 2], [1, MCS]]),
                            TT.mult,
                        )
                        dve.tensor_tensor(
                            DXY[c][:, h],
                            _ap(CRD[c], h * 2 * MCS, [[CS, M], [0, M], [1, CS]]),
                            _ap(CRD[c], h * 2 * MCS + MCS,
                                [[0, M], [CS, M], [1, CS]]),
                            TT.subtract,
                        )
                else:
                    dve.tensor_tensor(
                        _ap(CRD[c], 0, [[2 * MCS, 2], [MCS, 2], [1, MCS]]),
                        _ap(TRIG[c], 0, [[2 * MCS, 2], [MCS, 2], [1, MCS]]),
                        _ap(IN[c], 2 * MCS, [[0, 2], [MCS, 2], [1, MCS]]),
                        TT.mult,
                    )
                    for h in (1, 0):
                        dve.tensor_tensor(
                            DXY[c][:, h],
                            _ap(CRD[c], h * 2 * MCS, [[CS, M], [0, M], [1, CS]]),
                            _ap(CRD[c], h * 2 * MCS + MCS,
                                [[0, M], [CS, M], [1, CS]]),
                            TT.subtract,
                        )
                if sq_engine == "dve":
                    dve.tensor_tensor(SQ[c][:], DXY[c][:], DXY[c][:], TT.mult)

            # ---- squares; d2; sqrt in row groups (0-9 gates F01 pairs,
            #      10-19 gates F23 pairs, 20-24 gates arms) ----
            for c in range(nch):
                if sq_engine == "act":
                    nc.scalar.activation(SQ[c][:], DXY[c][:], AFT.Square)
                elif sq_engine == "split":
                    nc.scalar.activation(SQ[c][:, 1], DXY[c][:, 1], AFT.Square)
            for c in range(nch):
                if sq_engine == "split":
                    dve.tensor_tensor(
                        SQ[c][:, 0], DXY[c][:, 0], DXY[c][:, 0], TT.mult
                    )
                eng[d2_engine].tensor_tensor(
                    D2[c][:], SQ[c][:, 0], SQ[c][:, 1], TT.add
                )
            for r0, nrows in ((0, 10), (10, 10), (20, 5)):
                for c in range(nch):
                    nc.scalar.activation(
                        _ap(D, r0 * FS + c * CS, [[FS, nrows], [1, CS]]),
                        D2[c][:, r0:r0 + nrows], AFT.Sqrt,
                    )

            # ---- dense outer-sums: GT[f,a,b,s] = D[r0(f),a,s] + D[r1(f),b,s]
            # rows (0,1) for f=0 -> F01, rows (2,3) for f=1 -> F23. Split by
            # sample-half aligned to the sqrt chunks so each half starts as
            # soon as its chunk's rows are sqrted. ----
            HW_ = W // 2
            for f, (r0, r1) in enumerate(((0, 1), (2, 3))):
                for ho in (0, HW_):
                    dve.tensor_tensor(
                        _ap(GT, f * 25 * W + ho,
                            [[M * W, M], [W, M], [1, HW_]]),
                        _ap(D, r0 * 5 * FS + ho,
                            [[FS, M], [0, M], [1, HW_]]),
                        _ap(D, r1 * 5 * FS + ho,
                            [[0, M], [FS, M], [1, HW_]]),
                        TT.add,
                    )
            # compact dual-f triangle fold: FC[f,k] = min(G[a,b], G[b,a])
            for a in range(4):
                n = 4 - a
                dve.tensor_tensor(
                    _ap(FC, ROWSTART[a] * W, [[10 * W, 2], [W, n], [1, W]]),
                    _ap(GT, ((a * M) + a + 1) * W,
                        [[25 * W, 2], [W, n], [1, W]]),
                    _ap(GT, ((a + 1) * M + a) * W,
                        [[25 * W, 2], [M * W, n], [1, W]]),
                    TT.min,
                )

            # ---- arms: G3[T] (T-sorted 3-subsets) = min over c in T of
            # F23[T\c] + D4[c]; At computed first, consumed last, so a slow
            # engine there hides behind DVE's Ar/Aq. ----
            def f23c(idx, dims):
                return _ap(FC, (10 + idx) * W, dims)

            def d4(j, dims):
                return _ap(D, (20 + j) * FS, dims)

            e_t = eng[armt_engine]
            At_o = 0            # ARM slot offsets (elements)
            Ar_o = 10 * W
            # arm_t: At[T] = F23[{q,r}] + D4[t]
            e_t.tensor_tensor(
                _ap(ARM, At_o, [[W, 3], [1, W]]),
                f23c(0, [[0, 3], [1, W]]),
                d4(2, [[FS, 3], [1, W]]), TT.add)
            e_t.tensor_tensor(
                _ap(ARM, At_o + 3 * W, [[3 * W, 2], [W, 2], [1, W]]),
                f23c(1, [[3 * W, 2], [0, 2], [1, W]]),
                d4(3, [[0, 2], [FS, 2], [1, W]]), TT.add)
            e_t.tensor_tensor(
                _ap(ARM, At_o + 5 * W, [[3 * W, 2], [1, W]]),
                f23c(2, [[3 * W, 2], [1, W]]),
                d4(4, [[0, 2], [1, W]]), TT.add)
            e_t.tensor_tensor(
                _ap(ARM, At_o + 9 * W, [[0, 1], [1, W]]),
                f23c(7, [[0, 1], [1, W]]),
                d4(4, [[0, 1], [1, W]]), TT.add)
            # arm_r: Ar[T] = F23[{q,t}] + D4[r];  arm_q: Aq[T] = F23[{r,t}]
            # + D4[q]. Ar/Aq slot-0-2 and slot-9 ops read consecutive f23c
            # ranges (1-3 | 4-6 and 8 | 9), so each pair merges into one op
            # via an arm-dim stride.
            dve.tensor_tensor(
                _ap(ARM, Ar_o, [[10 * W, 2], [W, 3], [1, W]]),
                f23c(1, [[3 * W, 2], [W, 3], [1, W]]),
                d4(1, [[-FS, 2], [0, 3], [1, W]]), TT.add)
            dve.tensor_tensor(
                _ap(ARM, Ar_o + 3 * W, [[3 * W, 2], [W, 2], [1, W]]),
                f23c(2, [[3 * W, 2], [W, 2], [1, W]]),
                d4(2, [[0, 2], [0, 2], [1, W]]), TT.add)
            dve.tensor_tensor(
                _ap(ARM, Ar_o + 5 * W, [[3 * W, 2], [1, W]]),
                f23c(3, [[3 * W, 2], [1, W]]),
                d4(3, [[0, 2], [1, W]]), TT.add)
            dve.tensor_tensor(
                _ap(ARM, Ar_o + 9 * W, [[10 * W, 2], [1, W]]),
                f23c(8, [[W, 2], [1, W]]),
                d4(3, [[-FS, 2], [1, W]]), TT.add)
            # remaining Aq-only ops (slots 3,4,6,7 and 5,8)
            dve.tensor_tensor(
                _ap(ARM, Ar_o + 10 * W + 3 * W, [[3 * W, 2], [W, 2], [1, W]]),
                f23c(7, [[0, 2], [W, 2], [1, W]]),
                d4(0, [[FS, 2], [0, 2], [1, W]]), TT.add)
            dve.tensor_tensor(
                _ap(ARM, Ar_o + 10 * W + 5 * W, [[3 * W, 2], [1, W]]),
                f23c(9, [[0, 2], [1, W]]),
                d4(0, [[FS, 2], [1, W]]), TT.add)

            g3e = eng[g3_engine]
            g3e.tensor_tensor(G3[:], ARM[:, 1], ARM[:, 2], TT.min)
            g3e.tensor_tensor(G3[:], G3[:], ARM[:, 0], TT.min)

            # combine: ANS[k] = F01c[k] + G3[9-k] (complement reversal)
            dve.tensor_tensor(
                ANS[:], _ap(FC, 0, [[W, 10], [1, W]]),
                _ap(G3, 9 * W, [[-W, 10], [1, W]]), TT.add)
            # min tree over the 10 slots
            dve.tensor_tensor(T1[:], ANS[:, 0:5, :], ANS[:, 5:10, :], TT.min)
            dve.tensor_tensor(T2[:], T1[:, 0:2, :], T1[:, 2:4, :], TT.min)
            dve.tensor_tensor(T3[:], T2[:, 0:1, :], T2[:, 1:2, :], TT.min)
            if USE_TTR:
                # fused: RES = min(T3, T1[4]); PART = sum_s RES (one DVE op)
                dve.tensor_tensor_reduce(
                    RES[:, 0], T3[:, 0, :], T1[:, 4, :], 1.0, 0.0,
                    TT.min, TT.add, PART[:],
                )
            else:
                dve.tensor_tensor(RES[:, 0], T3[:, 0, :], T1[:, 4, :], TT.min)
                dve.tensor_reduce(
                    PART[:], _ap(RES, 0, [[1, W]]),
                    mybir.AxisListType.X, TT.add,
                )
            # collapse to one partition so the output DMA is a single-queue
            # 4-byte transfer (a [128,1] source fans out over 16 queues whose
            # completion crawl costs ~7us at the tail)
            gp.partition_all_reduce(
                PARTR[:], PART[:], 128, bass_isa.ReduceOp.add
            )
            nc.sync.dma_start(out=out_d[:], in_=PARTR[0:1, :])

    nc.compile()
    return nc


_CACHED_RUNNER = None


def _pack_inputs(ta, pa, td, pd, nch):
    """(N, M) f32 x4 -> (NCORES*P, nch, 2, 2, M, CS) f16, chunk-major."""
    CS = FS // nch
    out = np.empty((NCORES * P, nch, 2, 2, M, CS), np.float16)
    for k, (a, b) in enumerate(((ta, pa), (td, pd))):
        a5 = np.asarray(a, np.float32).reshape(NCORES * P, nch, CS, M)
        b5 = np.asarray(b, np.float32).reshape(NCORES * P, nch, CS, M)
        out[:, :, k, 0] = a5.transpose(0, 1, 3, 2)
        out[:, :, k, 1] = b5.transpose(0, 1, 3, 2)
    return out


def _make_runner():
    import jax
    from jax.sharding import Mesh, NamedSharding, PartitionSpec
    from jax.experimental.shard_map import shard_map
    from concourse.bass2jax import (
        _bass_exec_p, install_neuronx_cc_hook, partition_id_tensor,
    )

    nc = build_bass()
    install_neuronx_cc_hook()
    partition_name = nc.partition_id_tensor.name if nc.partition_id_tensor else None
    in_names, out_names, out_avals, zero_outs = [], [], [], []
    for alloc in nc.m.functions[0].allocations:
        if not isinstance(alloc, mybir.MemoryLocationSet):
            continue
        name = alloc.memorylocations[0].name
        if alloc.kind == "ExternalInput":
            if name != partition_name:
                in_names.append(name)
        elif alloc.kind == "ExternalOutput":
            shape = tuple(alloc.tensor_shape)
            dtype = mybir.dt.np(alloc.dtype)
            out_names.append(name)
            out_avals.append(jax.core.ShapedArray(shape, dtype))
            zero_outs.append(np.zeros(shape, dtype))
    n_params = len(in_names)
    all_in_names = in_names + out_names
    if partition_name is not None:
        all_in_names = all_in_names + [partition_name]

    def _body(*args):
        operands = list(args)
        if partition_name is not None:
            operands.append(partition_id_tensor())
        return tuple(_bass_exec_p.bind(
            *operands,
            out_avals=tuple(out_avals),
            in_names=tuple(all_in_names),
            out_names=tuple(out_names),
            lowering_input_output_aliases=(),
            sim_require_finite=True,
            sim_require_nnan=True,
            nc=nc,
        ))

    devices = jax.devices()[:NCORES]
    mesh = Mesh(np.asarray(devices), ("core",))
    in_specs = (PartitionSpec("core"),) * (n_params + len(out_names))
    out_specs = (PartitionSpec("core"),) * len(out_names)
    fn = jax.jit(
        shard_map(_body, mesh=mesh, in_specs=in_specs, out_specs=out_specs,
                  check_rep=False),
        keep_unused=True,
    )
    sharding = NamedSharding(mesh, PartitionSpec("core"))
    concat_zeros = [
        np.zeros((NCORES * z.shape[0], *z.shape[1:]), z.dtype) for z in zero_outs
    ]
    zeros_dev = [jax.device_put(z, sharding) for z in concat_zeros]

    def run(inputs_by_name):
        import jax as _jax
        args = [
            _jax.device_put(np.ascontiguousarray(inputs_by_name[nm]), sharding)
            for nm in in_names
        ]
        outs = fn(*args, *zeros_dev)
        return {nm: np.asarray(outs[i]) for i, nm in enumerate(out_names)}

    return run


def _input_map(ta, pa, td, pd):
    m = {"inp": _pack_inputs(ta, pa, td, pd, NCH)}
    if BIAS_DMA:
        m["hpi"] = np.tile(np.full((P, 1), HALF_PI, np.float32), (NCORES, 1))
    return m


def kernel(predictions_angle, targets_angle, predictions_distance, targets_distance):
    global _CACHED_RUNNER
    if _CACHED_RUNNER is None:
        _CACHED_RUNNER = _make_runner()
    out = _CACHED_RUNNER(_input_map(
        targets_angle, predictions_angle,
        targets_distance, predictions_distance,
    ))
    total = out["partials"].astype(np.float64).sum()
    return np.asarray(total / N / M, dtype=np.float32)
